# revision 26
# baseline (speedup 1.0000x reference)
"""Trainium2 Bass kernel for dilated multi-head self-attention with relative
positional embedding (B=8 data-parallel over 8 NeuronCores, 1 sample/core).

Pipeline per core:
  - q/k projections on PE (W-stationary). k^T is stored in a per-query-tile
    block layout kT2[(d), tile, 160]: cols 0:147 = the sliding key window for
    that tile (halos duplicated), cols 147:153 = Er columns for the head that
    owns the partition range. One fp16 matmul per (tile, head) then yields the
    full banded scores + q@Er in a single PSUM bank region.
  - v projection (x^T-stationary) producing V in (t x d) layout shifted by the
    window lower bound so out-matmul key segments are partition-aligned.
  - banded scores written fp16 to DRAM scratch; strided DMA readback of the
    per-row rolled window (row stride = pitch+1 turns the band diagonal into
    a column) -> compact (t x 6) taps + qEr.
  - compact softmax; normalized attention in fp32 (graded output) and fp16.
  - GPSIMD local_scatter rebuilds the banded attention matrix (t x k, fp16,
    zeros off-band), PE transposes -> (k x t), PE matmul against V segments.

Hardware constraint honored throughout: at most ONE matmul output region per
PSUM bank (multiple different-region matmuls into one bank crash the device);
same-region accumulation groups are fine.
"""

import math
import sys

sys.path.insert(0, "/opt/trn_rl_repo")

import numpy as np

from concourse import bass, bacc, mybir, tile
from concourse import bass_utils

DT = mybir.dt
F32 = DT.float32
F16 = DT.float16
I16 = DT.int16

B, T, D = 8, 4096, 512
H, HD = 8, 64
NT = T // 128              # 32 query tiles
NSC = 8                    # projection super-chunks
SCW = T // NSC             # 512
KBW = 160                  # kT2 block width: 147 band + 6 Er + pad
PITCH = 160                # DRAM band row pitch (elems)
ABW = 160                  # A-band width (128 + 32)
LAG = 6                    # tiles of PE-stream lag hiding the DMA roundtrip

_CACHE = {}


def _plan(layer):
    d = 2 ** layer
    r0 = int(round((1 + layer) / 11 * 10))
    cols = [j * d for j in range(5)] + [4 * d + r0]   # device tap cols (ascending)
    span = 4 * d + r0                                  # 19 @ layer=2
    sw = 128 + span                                    # 147 band columns
    padlo = 4 * d                                      # 16
    assert sw + 6 <= min(PITCH, KBW) and span < 32 and padlo < 128
    return d, r0, cols, span, sw, padlo


def _build(layer):
    d, r0, cols, span, SW, PADLO = _plan(layer)
    NVC = NT + 1                          # V chunks (33): rows r = t + PADLO
    ROLLW = span + 1                      # 20

    nc = bacc.Bacc("TRN2", target_bir_lowering=False, debug=False)

    # ---- DRAM I/O ----
    qT_d = nc.dram_tensor("qT", (D, T), F16, kind="ExternalInput")
    kTi_d = nc.dram_tensor("kTi", (D, T), F16, kind="ExternalInput")
    vT_d = nc.dram_tensor("vT", (D, T), F16, kind="ExternalInput")
    wq_d = nc.dram_tensor("WqTs", (128, 4, D), F16, kind="ExternalInput")
    wk_d = nc.dram_tensor("WkTs", (128, 4, D), F16, kind="ExternalInput")
    wv_d = nc.dram_tensor("WvTs", (128, 4, D), F16, kind="ExternalInput")
    bqc_d = nc.dram_tensor("bqc", (128, 4), F32, kind="ExternalInput")
    bkc_d = nc.dram_tensor("bkc", (128, 4), F32, kind="ExternalInput")
    bv1_d = nc.dram_tensor("bv1", (1, D), F16, kind="ExternalInput")
    eones_d = nc.dram_tensor("eones", (1, 3 * 128), F16, kind="ExternalInput")
    erk_d = nc.dram_tensor("ErK", (128, 4, NT, 6), F16, kind="ExternalInput")
    idx_d = nc.dram_tensor("idx16", (128, 48), I16, kind="ExternalInput")
    idf_d = nc.dram_tensor("idf16", (128, 128), F16, kind="ExternalInput")
    rep_d = nc.dram_tensor("rep16", (32, 128), F16, kind="ExternalInput")

    out_d = nc.dram_tensor("out", (T, D), F32, kind="ExternalOutput")
    attnc_d = nc.dram_tensor("attnc", (NT, 128, 48), F32, kind="ExternalOutput")

    band_d = nc.dram_tensor("bandscr", (NT, H, 128, PITCH), F16, kind="Internal")
    warm_d = nc.dram_tensor("warmscr", (128, 64), F16, kind="Internal")

    scale = 1.0 / math.sqrt(HD)

    with tile.TileContext(nc) as tc:
        with (
            tc.tile_pool(name="const", bufs=1) as cpool,
            tc.tile_pool(name="persist", bufs=1) as ppool,
            tc.tile_pool(name="qring", bufs=8) as qpool,
            tc.tile_pool(name="xin", bufs=2) as xpool,
            tc.tile_pool(name="vin", bufs=2) as vpool,
            tc.tile_pool(name="pA", bufs=2, space="PSUM") as papool,
            tc.tile_pool(name="pB", bufs=2, space="PSUM") as pbpool,
            tc.tile_pool(name="work", bufs=2) as wpool,
        ):
            # ---- HAM warmup: ~9us of dense dummy matmuls, no DMA deps ----
            wrm = cpool.tile([128, 512], F16, tag="wrm")
            nc.vector.memset(wrm[:], 0.0)
            pwarm = papool.tile([128, 2, 512], F32, tag="pa", name="pwarm")
            for w_ in range(64):
                nc.tensor.matmul(pwarm[:, 0, :], wrm[:, 0:128], wrm[:],
                                 start=(w_ == 0), stop=(w_ == 63))
            wrm2 = cpool.tile([128, 64], F16, tag="wrm2")
            nc.scalar.copy(wrm2[:], pwarm[:, 0, 0:64])
            nc.sync.dma_start(warm_d.ap(), wrm2[:])

            # ---- constants ----
            wq_s = cpool.tile([128, 4, D], F16, tag="wq")
            wk_s = cpool.tile([128, 4, D], F16, tag="wk")
            wv_s = cpool.tile([128, 4, D], F16, tag="wv")
            bqc_s = cpool.tile([128, 4], F32, tag="bqc")
            bkc_s = cpool.tile([128, 4], F32, tag="bkc")
            bv1_s = cpool.tile([1, D], F16, tag="bv1")
            eones_s = cpool.tile([1, 3 * 128], F16, tag="eones")
            idx_s = cpool.tile([128, 48], I16, tag="idx")
            idf_s = cpool.tile([128, 128], F16, tag="idf")
            rep_s = cpool.tile([32, 128], F16, tag="rep")
            for t_, d_ in ((wk_s, wk_d), (bkc_s, bkc_d), (wq_s, wq_d),
                           (bqc_s, bqc_d)):
                nc.sync.dma_start(t_[:], d_.ap())

            # ---- persistent activations ----
            kT2 = ppool.tile([128, 4, NT, KBW], F16, tag="kT2")
            v_s = ppool.tile([128, NVC, D], F16, tag="V")
            # zero edges: block 0 left halo, block NT-1 right halo, pad col
            nc.vector.memset(kT2[:, :, 0, 0:PADLO], 0.0)
            nc.vector.memset(kT2[:, :, NT - 1, PADLO + 128:SW], 0.0)

            # Er columns (constant per (partition-range, head))
            for dp in range(4):
                nc.sync.dma_start(kT2[:, dp, :, SW:SW + 6],
                                  erk_d.ap()[:, dp])

            def emit_late_consts():
                for t_, d_ in ((wv_s, wv_d), (bv1_s, bv1_d),
                               (eones_s, eones_d), (idx_s, idx_d),
                               (idf_s, idf_d), (rep_s, rep_d)):
                    nc.sync.dma_start(t_[:], d_.ap())

            qtiles = [None] * NSC
            v_emitted = 0

            def emit_qk_proj(proj, sc):
                xd = qT_d if proj == 0 else kTi_d
                w_s = wq_s if proj == 0 else wk_s
                bc_s = bqc_s if proj == 0 else bkc_s
                xin = xpool.tile([128, 4, SCW], F16, tag="xin",
                                 name=f"xin_{proj}_{sc}")
                nc.sync.dma_start(xin[:], bass.AP(
                    tensor=xd, offset=sc * SCW,
                    ap=[[T, 128], [128 * T, 4], [1, SCW]]))
                if proj == 0:
                    qt = qpool.tile([128, 4, SCW], F16, tag="q", name=f"q_{sc}")
                    qtiles[sc] = qt
                for oc in range(4):
                    ps = pbpool.tile([128, 2, SCW], F32, tag="pb",
                                     name=f"pspj_{proj}_{sc}_{oc}")
                    for kc in range(4):
                        nc.tensor.matmul(
                            ps[:, 0, :], w_s[:, kc, oc * 128:(oc + 1) * 128],
                            xin[:, kc, :], start=(kc == 0), stop=(kc == 3))
                    if proj == 0:
                        nc.vector.tensor_scalar_add(
                            qtiles[sc][:, oc, :], ps[:, 0, :], bc_s[:, oc:oc + 1])
                    else:
                        # scatter into kT2 block layout (f16, bias fused)
                        j0 = 4 * sc
                        psb = ps[:, 0, :].rearrange("p (b c) -> p b c", b=4)
                        nc.vector.tensor_scalar_add(
                            kT2[:, oc, j0:j0 + 4, PADLO:PADLO + 128],
                            psb, bc_s[:, oc:oc + 1])
                        # left halos -> blocks j0+1 .. j0+4
                        nb = 3 if sc == NSC - 1 else 4
                        nc.vector.tensor_scalar_add(
                            kT2[:, oc, j0 + 1:j0 + 1 + nb, 0:PADLO],
                            psb[:, 0:nb, 128 - PADLO:128],
                            bc_s[:, oc:oc + 1])
                        # right halos -> blocks j0-1 .. j0+2
                        hb = span - PADLO      # 3 cols
                        if sc == 0:
                            nc.vector.tensor_scalar_add(
                                kT2[:, oc, 0:3, PADLO + 128:SW],
                                psb[:, 1:4, 0:hb], bc_s[:, oc:oc + 1])
                        else:
                            nc.vector.tensor_scalar_add(
                                kT2[:, oc, j0 - 1:j0 + 3, PADLO + 128:SW],
                                psb[:, 0:4, 0:hb], bc_s[:, oc:oc + 1])

            def emit_v_proj(j):
                tlo = j * 128 - PADLO
                vin = vpool.tile([128, 4, 128], F16, tag="vin", name=f"vin_{j}")
                lo_clip = max(0, -tlo)
                hi_clip = max(0, tlo + 128 - T)
                if lo_clip or hi_clip:
                    nc.vector.memset(vin[:], 0.0)
                nrow = 128 - lo_clip - hi_clip
                nc.sync.dma_start(
                    vin[:, :, lo_clip:lo_clip + nrow],
                    bass.AP(tensor=vT_d, offset=tlo + lo_clip,
                            ap=[[T, 128], [128 * T, 4], [1, nrow]]))
                pv = pbpool.tile([128, 2, D], F32, tag="pb", name=f"pspv_{j}")
                for kc in range(4):
                    nc.tensor.matmul(
                        pv[:, 0, :], vin[:, kc, :], wv_s[:, kc, :],
                        start=(kc == 0), stop=False)
                ev = 0 if j == 0 else (2 if j == NVC - 1 else 1)
                nc.tensor.matmul(
                    pv[:, 0, :], eones_s[0:1, ev * 128:(ev + 1) * 128], bv1_s[:],
                    start=False, stop=True)
                nc.scalar.copy(v_s[:, j, :], pv[:, 0, :])

            abands = [None] * NT

            def emit_attn_front(i):
                t0 = 128 * i
                sc, scoff = i // 4, (i % 4) * 128
                qt = qtiles[sc]
                band = wpool.tile([128, H, PITCH], F16, tag="band",
                                  name=f"band_{i}")
                for g in range(4):          # head pairs; 1 matmul per bank
                    ps = papool.tile([128, 2, 512], F32, tag="pa",
                                     name=f"psbd_{i}_{g}")
                    for hh in range(2):
                        h = 2 * g + hh
                        dp, bp = h // 2, (h % 2) * 64
                        nc.tensor.matmul(
                            ps[:, hh, 0:SW + 6],
                            qt[bp:bp + 64, dp, scoff:scoff + 128],
                            kT2[bp:bp + 64, dp, i, 0:SW + 6],
                            start=True, stop=True)
                    if g % 2 == 0:
                        nc.scalar.copy(band[:, 2 * g:2 * g + 2, 0:SW + 6],
                                       ps[:, :, 0:SW + 6])
                    else:
                        nc.vector.tensor_copy(band[:, 2 * g:2 * g + 2, 0:SW + 6],
                                              ps[:, :, 0:SW + 6])
                nc.sync.dma_start(
                    bass.AP(tensor=band_d, offset=i * (H * 128 * PITCH),
                            ap=[[PITCH, 128], [128 * PITCH, H], [1, SW + 6]]),
                    band[:, :, 0:SW + 6])

                # rolled readback: roll[p,h,c] = band[i,h,p,p+c]
                roll = wpool.tile([128, H, ROLLW], F16, tag="roll",
                                  name=f"roll_{i}")
                base = i * (H * 128 * PITCH)
                nc.sync.dma_start(roll[:], bass.AP(
                    tensor=band_d, offset=base,
                    ap=[[PITCH + 1, 128], [128 * PITCH, H], [1, ROLLW]]))
                er2 = wpool.tile([128, H, 6], F16, tag="er2", name=f"er2_{i}")
                nc.sync.dma_start(er2[:], bass.AP(
                    tensor=band_d, offset=base + SW,
                    ap=[[PITCH, 128], [128 * PITCH, H], [1, 6]]))

                # compact softmax
                sc48 = wpool.tile([128, H, 6], F32, tag="sc48", name=f"sc48_{i}")
                nc.vector.tensor_add(
                    sc48[:, :, 0:5], roll[:, :, 0:4 * d + 1:d], er2[:, :, 0:5])
                nc.vector.tensor_add(
                    sc48[:, :, 5:6], roll[:, :, 4 * d + r0:4 * d + r0 + 1],
                    er2[:, :, 5:6])
                e48 = wpool.tile([128, H, 6], F32, tag="e48", name=f"e48_{i}")
                nc.scalar.activation(
                    e48[:], sc48[:], mybir.ActivationFunctionType.Exp,
                    scale=scale)
                sums = wpool.tile([128, H, 1], F32, tag="sums", name=f"sums_{i}")
                nc.vector.reduce_sum(sums[:, :, 0], e48[:],
                                     axis=mybir.AxisListType.X)
                n8 = wpool.tile([128, H, 1], F32, tag="n8", name=f"n8_{i}")
                nc.vector.reciprocal(n8[:], sums[:])
                a48 = wpool.tile([128, H, 6], F32, tag="a48", name=f"a48_{i}")
                nc.vector.tensor_mul(a48[:], e48[:],
                                     n8[:].broadcast_to([128, H, 6]))
                nc.sync.dma_start(attnc_d.ap()[i],
                                  a48[:].rearrange("p h c -> p (h c)"))
                a48h = wpool.tile([128, 48], F16, tag="a48h", name=f"a48h_{i}")
                nc.vector.tensor_copy(a48h[:],
                                      a48[:].rearrange("p h c -> p (h c)"))

                # scatter into banded A (t x k), fp16
                aband = wpool.tile([128, H * ABW], F16, tag="aband",
                                   name=f"aband_{i}", bufs=LAG + 2)
                abands[i] = aband
                nc.gpsimd.local_scatter(
                    aband[:], a48h[:], idx_s[:],
                    channels=128, num_elems=H * ABW, num_idxs=48)

            def emit_attn_back(i):
                t0 = 128 * i
                aband = abands[i]
                # transposes -> (k x t); one transpose per PSUM bank.
                # hi parts per head; lo parts packed 4-heads-per-transpose.
                athi = wpool.tile([128, H, 128], F16, tag="athi",
                                  name=f"athi_{i}")
                atlo4 = wpool.tile([128, 2, 128], F16, tag="atlo4",
                                   name=f"atlo4_{i}")
                for r in range(4):          # head pairs: transposes
                    h0, h1 = 2 * r, 2 * r + 1
                    trhi = papool.tile([128, 2, 1024], F16, tag="pa",
                                       name=f"trhi_{i}_{r}")
                    for hh, h in ((0, h0), (1, h1)):
                        nc.tensor.transpose(
                            trhi[:, hh, 0:128],
                            aband[:, h * 128:h * 128 + 128], idf_s[:])
                    nc.vector.tensor_copy(athi[:, h0:h0 + 2, :],
                                          trhi[:, :, 0:128])
                trlo = papool.tile([128, 2, 1024], F16, tag="pa",
                                   name=f"trlo_{i}")
                for g_ in range(2):
                    nc.tensor.transpose(
                        trlo[:, g_, 0:128],
                        aband[:, 1024 + g_ * 128:1024 + g_ * 128 + 128],
                        idf_s[:])
                nc.scalar.copy(atlo4[:], trlo[:, :, 0:128])
                # replicate V lo rows to all four 32-partition groups
                pvr = pbpool.tile([128, 2, D], F32, tag="pb",
                                  name=f"pvr_{i}")
                nc.tensor.matmul(pvr[:, 0, :], rep_s[:],
                                 v_s[0:32, i + 1, :], start=True, stop=True)
                vrep = wpool.tile([128, D], F16, tag="vrep", name=f"vrep_{i}")
                nc.vector.tensor_copy(vrep[:], pvr[:, 0, :])

                osb = wpool.tile([128, D], F32, tag="osb", name=f"osb_{i}")
                for r in range(4):          # head pairs: out matmuls
                    h0, h1 = 2 * r, 2 * r + 1
                    po = pbpool.tile([128, 2, D], F32, tag="pb",
                                     name=f"po_{i}_{r}")
                    for hh, h in ((0, h0), (1, h1)):
                        g_, sl = h // 4, h % 4
                        nc.tensor.matmul(
                            po[:, hh, 0:HD], athi[:, h, :],
                            v_s[:, i, h * HD:(h + 1) * HD],
                            start=True, stop=False)
                        nc.tensor.matmul(
                            po[:, hh, 0:HD],
                            atlo4[32 * sl:32 * sl + 32, g_, :],
                            vrep[32 * sl:32 * sl + 32, h * HD:(h + 1) * HD],
                            start=False, stop=True,
                            tile_position=(32 * sl, 0))
                    if r % 2 == 0:
                        nc.vector.tensor_copy(
                            osb[:, h0 * HD:(h0 + 2) * HD].rearrange(
                                "p (b c) -> p b c", b=2),
                            po[:, :, 0:HD])
                    else:
                        nc.scalar.copy(
                            osb[:, h0 * HD:(h0 + 2) * HD].rearrange(
                                "p (b c) -> p b c", b=2),
                            po[:, :, 0:HD])
                nc.sync.dma_start(out_d.ap()[t0:t0 + 128, :], osb[:])

            # ---- dense projection phase (keeps the PE clock warm), then
            # ---- the pipelined attention phase
            for sc in range(NSC):
                emit_qk_proj(1, sc)
                if sc == 0:
                    emit_late_consts()
                emit_qk_proj(0, sc)
            while v_emitted <= NVC - 1:
                emit_v_proj(v_emitted)
                v_emitted += 1
            for i in range(NT):
                emit_attn_front(i)
                if i >= LAG:
                    emit_attn_back(i - LAG)
            for i in range(NT - LAG, NT):
                emit_attn_back(i)

    nc.compile()
    return nc


def _host_prepare(inputs, layer):
    d, r0, cols, span, SW, PADLO = _plan(layer)
    q = np.ascontiguousarray(np.asarray(inputs["query"], np.float32))
    k = np.ascontiguousarray(np.asarray(inputs["key"], np.float32))
    v = np.ascontiguousarray(np.asarray(inputs["value"], np.float32))
    Wq = np.asarray(inputs["Wq"], np.float32)
    Wk = np.asarray(inputs["Wk"], np.float32)
    Wv = np.asarray(inputs["Wv"], np.float32)
    Er = np.asarray(inputs["Er"], np.float32)

    def wstack(Wm):
        return np.ascontiguousarray(
            Wm.T.reshape(4, 128, D).transpose(1, 0, 2).astype(np.float16))

    consts = {
        "WqTs": wstack(Wq), "WkTs": wstack(Wk), "WvTs": wstack(Wv),
        "bqc": np.ascontiguousarray(
            np.asarray(inputs["bq"], np.float32).reshape(4, 128).T),
        "bkc": np.ascontiguousarray(
            np.asarray(inputs["bk"], np.float32).reshape(4, 128).T),
        "bv1": np.asarray(inputs["bv"], np.float16).reshape(1, D),
    }
    eones = np.ones((1, 3 * 128), np.float16)
    eones[0, 0:PADLO] = 0.0
    NVC = NT + 1
    tlo_last = (NVC - 1) * 128 - PADLO
    nvalid = max(0, T - tlo_last)
    eones[0, 2 * 128:] = 0.0
    eones[0, 2 * 128:2 * 128 + nvalid] = 1.0
    consts["eones"] = eones

    erk = np.zeros((128, 4, NT, 6), np.float16)
    for h in range(H):
        dp, half = h // 2, h % 2
        blk = Er[h, :, ::-1].astype(np.float16)        # (64, 6) device order
        erk[half * 64:(half + 1) * 64, dp, :, :] = blk[:, None, :]
    consts["ErK"] = erk

    idx = np.zeros((128, 48), np.int16)
    for p in range(128):
        for h in range(H):
            for j in range(6):
                c = p + cols[j]
                if c < 128:
                    dst = h * 128 + c
                else:
                    dst = 1024 + (h // 4) * 128 + (h % 4) * 32 + (c - 128)
                idx[p, h * 6 + j] = dst
    consts["idx16"] = idx
    rep = np.zeros((32, 128), np.float16)
    for p in range(32):
        rep[p, p::32] = 1.0
    consts["rep16"] = rep
    consts["idf16"] = np.eye(128, dtype=np.float16)

    in_maps = []
    for c in range(B):
        m = dict(consts)
        m["qT"] = np.ascontiguousarray(q[c].T.astype(np.float16))
        m["kTi"] = np.ascontiguousarray(k[c].T.astype(np.float16))
        m["vT"] = np.ascontiguousarray(v[c].T.astype(np.float16))
        in_maps.append(m)
    return in_maps


def kernel(**inputs):
    layer = int(np.asarray(inputs["layer"]))
    if layer not in _CACHE:
        _CACHE[layer] = _build(layer)
    nc = _CACHE[layer]
    in_maps = _host_prepare(inputs, layer)
    res = bass_utils.run_bass_kernel_spmd(nc, in_maps, core_ids=list(range(B)))
    outs, attns = [], []
    for c in range(B):
        r = res.results[c]
        outs.append(np.asarray(r["out"], np.float32))
        ac = np.asarray(r["attnc"], np.float32).reshape(T, H, 6)
        attns.append(ac.transpose(1, 0, 2)[:, :, ::-1][:, :, None, :])
    return np.stack(outs), np.stack(attns)


# revision 27
# speedup vs baseline: 1.0620x; 1.0620x over previous
"""Trainium2 Bass kernel for dilated multi-head self-attention with relative
positional embedding (B=8 data-parallel over 8 NeuronCores, 1 sample/core).

Pipeline per core:
  - q/k projections on PE (W-stationary). k^T is stored in a per-query-tile
    block layout kT2[(d), tile, 160]: cols 0:147 = the sliding key window for
    that tile (halos duplicated), cols 147:153 = Er columns for the head that
    owns the partition range. One fp16 matmul per (tile, head) then yields the
    full banded scores + q@Er in a single PSUM bank region.
  - v projection (x^T-stationary) producing V in (t x d) layout shifted by the
    window lower bound so out-matmul key segments are partition-aligned.
  - banded scores written fp16 to DRAM scratch; strided DMA readback of the
    per-row rolled window (row stride = pitch+1 turns the band diagonal into
    a column) -> compact (t x 6) taps + qEr.
  - compact softmax; normalized attention in fp32 (graded output) and fp16.
  - GPSIMD local_scatter rebuilds the banded attention matrix (t x k, fp16,
    zeros off-band), PE transposes -> (k x t), PE matmul against V segments.

Hardware constraint honored throughout: at most ONE matmul output region per
PSUM bank (multiple different-region matmuls into one bank crash the device);
same-region accumulation groups are fine.
"""

import math
import sys

sys.path.insert(0, "/opt/trn_rl_repo")

import numpy as np

from concourse import bass, bacc, mybir, tile
from concourse import bass_utils

DT = mybir.dt
F32 = DT.float32
F16 = DT.float16
I16 = DT.int16

B, T, D = 8, 4096, 512
H, HD = 8, 64
NT = T // 128              # 32 query tiles
NSC = 8                    # projection super-chunks
SCW = T // NSC             # 512
KBW = 160                  # kT2 block width: 147 band + 6 Er + pad
PITCH = 160                # DRAM band row pitch (elems)
ABW = 160                  # A-band width (128 + 32)
LAG = 6                    # tiles of PE-stream lag hiding the DMA roundtrip

_CACHE = {}


def _plan(layer):
    d = 2 ** layer
    r0 = int(round((1 + layer) / 11 * 10))
    cols = [j * d for j in range(5)] + [4 * d + r0]   # device tap cols (ascending)
    span = 4 * d + r0                                  # 19 @ layer=2
    sw = 128 + span                                    # 147 band columns
    padlo = 4 * d                                      # 16
    assert sw + 6 <= min(PITCH, KBW) and span < 32 and padlo < 128
    return d, r0, cols, span, sw, padlo


def _build(layer):
    d, r0, cols, span, SW, PADLO = _plan(layer)
    NVC = NT + 1                          # V chunks (33): rows r = t + PADLO
    ROLLW = span + 1                      # 20

    nc = bacc.Bacc("TRN2", target_bir_lowering=False, debug=False)

    # ---- DRAM I/O ----
    qT_d = nc.dram_tensor("qT", (D, T), F16, kind="ExternalInput")
    kTi_d = nc.dram_tensor("kTi", (D, T), F16, kind="ExternalInput")
    vT_d = nc.dram_tensor("vT", (D, T), F16, kind="ExternalInput")
    wq_d = nc.dram_tensor("WqTs", (128, 4, D), F16, kind="ExternalInput")
    wk_d = nc.dram_tensor("WkTs", (128, 4, D), F16, kind="ExternalInput")
    wv_d = nc.dram_tensor("WvTs", (128, 4, D), F16, kind="ExternalInput")
    bqc_d = nc.dram_tensor("bqc", (128, 4), F32, kind="ExternalInput")
    bkc_d = nc.dram_tensor("bkc", (128, 4), F32, kind="ExternalInput")
    bv1_d = nc.dram_tensor("bv1", (1, D), F16, kind="ExternalInput")
    eones_d = nc.dram_tensor("eones", (1, 3 * 128), F16, kind="ExternalInput")
    erk_d = nc.dram_tensor("ErK", (128, 4, NT, 6), F16, kind="ExternalInput")
    idx_d = nc.dram_tensor("idx16", (128, 48), I16, kind="ExternalInput")
    idf_d = nc.dram_tensor("idf16", (128, 128), F16, kind="ExternalInput")
    rep_d = nc.dram_tensor("rep16", (32, 128), F16, kind="ExternalInput")

    out_d = nc.dram_tensor("out", (T, D), F32, kind="ExternalOutput")
    attnc_d = nc.dram_tensor("attnc", (NT, 128, 48), F32, kind="ExternalOutput")

    band_d = nc.dram_tensor("bandscr", (NT, H, 128, PITCH), F16, kind="Internal")
    warm_d = nc.dram_tensor("warmscr", (128, 64), F16, kind="Internal")

    scale = 1.0 / math.sqrt(HD)

    with tile.TileContext(nc) as tc:
        with (
            tc.tile_pool(name="const", bufs=1) as cpool,
            tc.tile_pool(name="persist", bufs=1) as ppool,
            tc.tile_pool(name="qring", bufs=2) as qpool,
            tc.tile_pool(name="xin", bufs=2) as xpool,
            tc.tile_pool(name="vin", bufs=2) as vpool,
            tc.tile_pool(name="pA", bufs=2, space="PSUM") as papool,
            tc.tile_pool(name="pB", bufs=2, space="PSUM") as pbpool,
            tc.tile_pool(name="work", bufs=2) as wpool,
        ):
            # ---- HAM warmup: ~9us of dense dummy matmuls, no DMA deps ----
            wrm = cpool.tile([128, 512], F16, tag="wrm")
            nc.vector.memset(wrm[:], 0.0)
            pwarm = papool.tile([128, 2, 512], F32, tag="pa", name="pwarm")
            for w_ in range(64):
                nc.tensor.matmul(pwarm[:, 0, :], wrm[:, 0:128], wrm[:],
                                 start=(w_ == 0), stop=(w_ == 63))
            wrm2 = cpool.tile([128, 64], F16, tag="wrm2")
            nc.scalar.copy(wrm2[:], pwarm[:, 0, 0:64])
            nc.sync.dma_start(warm_d.ap(), wrm2[:])

            # ---- constants ----
            wq_s = cpool.tile([128, 4, D], F16, tag="wq")
            wk_s = cpool.tile([128, 4, D], F16, tag="wk")
            wv_s = cpool.tile([128, 4, D], F16, tag="wv")
            bqc_s = cpool.tile([128, 4], F32, tag="bqc")
            bkc_s = cpool.tile([128, 4], F32, tag="bkc")
            bv1_s = cpool.tile([1, D], F16, tag="bv1")
            eones_s = cpool.tile([1, 3 * 128], F16, tag="eones")
            idx_s = cpool.tile([128, 48], I16, tag="idx")
            idf_s = cpool.tile([128, 128], F16, tag="idf")
            rep_s = cpool.tile([32, 128], F16, tag="rep")
            for t_, d_ in ((wk_s, wk_d), (bkc_s, bkc_d), (wq_s, wq_d),
                           (bqc_s, bqc_d)):
                nc.sync.dma_start(t_[:], d_.ap())

            # ---- persistent activations ----
            kT2 = ppool.tile([128, 4, NT, KBW], F16, tag="kT2")
            v_s = ppool.tile([128, NVC, D], F16, tag="V")
            # zero edges: block 0 left halo, block NT-1 right halo, pad col
            nc.vector.memset(kT2[:, :, 0, 0:PADLO], 0.0)
            nc.vector.memset(kT2[:, :, NT - 1, PADLO + 128:SW], 0.0)

            # Er columns (constant per (partition-range, head))
            for dp in range(4):
                nc.sync.dma_start(kT2[:, dp, :, SW:SW + 6],
                                  erk_d.ap()[:, dp])

            def emit_late_consts():
                for t_, d_ in ((wv_s, wv_d), (bv1_s, bv1_d),
                               (eones_s, eones_d), (idx_s, idx_d),
                               (idf_s, idf_d), (rep_s, rep_d)):
                    nc.sync.dma_start(t_[:], d_.ap())

            qtiles = [None] * NSC
            v_emitted = 0

            def emit_qk_proj(proj, sc):
                xd = qT_d if proj == 0 else kTi_d
                w_s = wq_s if proj == 0 else wk_s
                bc_s = bqc_s if proj == 0 else bkc_s
                xin = xpool.tile([128, 4, SCW], F16, tag="xin",
                                 name=f"xin_{proj}_{sc}")
                nc.sync.dma_start(xin[:], bass.AP(
                    tensor=xd, offset=sc * SCW,
                    ap=[[T, 128], [128 * T, 4], [1, SCW]]))
                if proj == 0:
                    qt = qpool.tile([128, 4, SCW], F16, tag="q", name=f"q_{sc}")
                    qtiles[sc] = qt
                for oc in range(4):
                    ps = pbpool.tile([128, 2, SCW], F32, tag="pb",
                                     name=f"pspj_{proj}_{sc}_{oc}")
                    for kc in range(4):
                        nc.tensor.matmul(
                            ps[:, 0, :], w_s[:, kc, oc * 128:(oc + 1) * 128],
                            xin[:, kc, :], start=(kc == 0), stop=(kc == 3))
                    if proj == 0:
                        nc.vector.tensor_scalar_add(
                            qtiles[sc][:, oc, :], ps[:, 0, :], bc_s[:, oc:oc + 1])
                    else:
                        # scatter into kT2 block layout (f16, bias fused)
                        j0 = 4 * sc
                        psb = ps[:, 0, :].rearrange("p (b c) -> p b c", b=4)
                        nc.vector.tensor_scalar_add(
                            kT2[:, oc, j0:j0 + 4, PADLO:PADLO + 128],
                            psb, bc_s[:, oc:oc + 1])
                        # left halos -> blocks j0+1 .. j0+4
                        nb = 3 if sc == NSC - 1 else 4
                        nc.vector.tensor_scalar_add(
                            kT2[:, oc, j0 + 1:j0 + 1 + nb, 0:PADLO],
                            psb[:, 0:nb, 128 - PADLO:128],
                            bc_s[:, oc:oc + 1])
                        # right halos -> blocks j0-1 .. j0+2
                        hb = span - PADLO      # 3 cols
                        if sc == 0:
                            nc.vector.tensor_scalar_add(
                                kT2[:, oc, 0:3, PADLO + 128:SW],
                                psb[:, 1:4, 0:hb], bc_s[:, oc:oc + 1])
                        else:
                            nc.vector.tensor_scalar_add(
                                kT2[:, oc, j0 - 1:j0 + 3, PADLO + 128:SW],
                                psb[:, 0:4, 0:hb], bc_s[:, oc:oc + 1])

            def emit_v_proj(j):
                tlo = j * 128 - PADLO
                vin = vpool.tile([128, 4, 128], F16, tag="vin", name=f"vin_{j}")
                lo_clip = max(0, -tlo)
                hi_clip = max(0, tlo + 128 - T)
                if lo_clip or hi_clip:
                    nc.vector.memset(vin[:], 0.0)
                nrow = 128 - lo_clip - hi_clip
                nc.sync.dma_start(
                    vin[:, :, lo_clip:lo_clip + nrow],
                    bass.AP(tensor=vT_d, offset=tlo + lo_clip,
                            ap=[[T, 128], [128 * T, 4], [1, nrow]]))
                pv = pbpool.tile([128, 2, D], F32, tag="pb", name=f"pspv_{j}")
                for kc in range(4):
                    nc.tensor.matmul(
                        pv[:, 0, :], vin[:, kc, :], wv_s[:, kc, :],
                        start=(kc == 0), stop=False)
                ev = 0 if j == 0 else (2 if j == NVC - 1 else 1)
                nc.tensor.matmul(
                    pv[:, 0, :], eones_s[0:1, ev * 128:(ev + 1) * 128], bv1_s[:],
                    start=False, stop=True)
                nc.scalar.copy(v_s[:, j, :], pv[:, 0, :])

            abands = [None] * NT

            def emit_attn_front(i):
                t0 = 128 * i
                sc, scoff = i // 4, (i % 4) * 128
                qt = qtiles[sc]
                band = wpool.tile([128, H, PITCH], F16, tag="band",
                                  name=f"band_{i}")
                for g in range(4):          # head pairs; 1 matmul per bank
                    ps = papool.tile([128, 2, 512], F32, tag="pa",
                                     name=f"psbd_{i}_{g}")
                    for hh in range(2):
                        h = 2 * g + hh
                        dp, bp = h // 2, (h % 2) * 64
                        nc.tensor.matmul(
                            ps[:, hh, 0:SW + 6],
                            qt[bp:bp + 64, dp, scoff:scoff + 128],
                            kT2[bp:bp + 64, dp, i, 0:SW + 6],
                            start=True, stop=True)
                    if g % 2 == 0:
                        nc.scalar.copy(band[:, 2 * g:2 * g + 2, 0:SW + 6],
                                       ps[:, :, 0:SW + 6])
                    else:
                        nc.vector.tensor_copy(band[:, 2 * g:2 * g + 2, 0:SW + 6],
                                              ps[:, :, 0:SW + 6])
                nc.sync.dma_start(
                    bass.AP(tensor=band_d, offset=i * (H * 128 * PITCH),
                            ap=[[PITCH, 128], [128 * PITCH, H], [1, SW + 6]]),
                    band[:, :, 0:SW + 6])

                # rolled readback: roll[p,h,c] = band[i,h,p,p+c]
                roll = wpool.tile([128, H, ROLLW], F16, tag="roll",
                                  name=f"roll_{i}")
                base = i * (H * 128 * PITCH)
                nc.sync.dma_start(roll[:], bass.AP(
                    tensor=band_d, offset=base,
                    ap=[[PITCH + 1, 128], [128 * PITCH, H], [1, ROLLW]]))
                er2 = wpool.tile([128, H, 6], F16, tag="er2", name=f"er2_{i}")
                nc.sync.dma_start(er2[:], bass.AP(
                    tensor=band_d, offset=base + SW,
                    ap=[[PITCH, 128], [128 * PITCH, H], [1, 6]]))

                # compact softmax
                sc48 = wpool.tile([128, H, 6], F32, tag="sc48", name=f"sc48_{i}")
                nc.vector.tensor_add(
                    sc48[:, :, 0:5], roll[:, :, 0:4 * d + 1:d], er2[:, :, 0:5])
                nc.vector.tensor_add(
                    sc48[:, :, 5:6], roll[:, :, 4 * d + r0:4 * d + r0 + 1],
                    er2[:, :, 5:6])
                e48 = wpool.tile([128, H, 6], F32, tag="e48", name=f"e48_{i}")
                nc.scalar.activation(
                    e48[:], sc48[:], mybir.ActivationFunctionType.Exp,
                    scale=scale)
                sums = wpool.tile([128, H, 1], F32, tag="sums", name=f"sums_{i}")
                nc.vector.reduce_sum(sums[:, :, 0], e48[:],
                                     axis=mybir.AxisListType.X)
                n8 = wpool.tile([128, H, 1], F32, tag="n8", name=f"n8_{i}")
                nc.vector.reciprocal(n8[:], sums[:])
                a48 = wpool.tile([128, H, 6], F32, tag="a48", name=f"a48_{i}")
                nc.vector.tensor_mul(a48[:], e48[:],
                                     n8[:].broadcast_to([128, H, 6]))
                nc.sync.dma_start(attnc_d.ap()[i],
                                  a48[:].rearrange("p h c -> p (h c)"))
                a48h = wpool.tile([128, 48], F16, tag="a48h", name=f"a48h_{i}")
                nc.vector.tensor_copy(a48h[:],
                                      a48[:].rearrange("p h c -> p (h c)"))

                # scatter into banded A (t x k), fp16
                aband = wpool.tile([128, H * ABW], F16, tag="aband",
                                   name=f"aband_{i}", bufs=LAG + 2)
                abands[i] = aband
                nc.gpsimd.local_scatter(
                    aband[:], a48h[:], idx_s[:],
                    channels=128, num_elems=H * ABW, num_idxs=48)

            def emit_attn_back(i):
                t0 = 128 * i
                aband = abands[i]
                # transposes -> (k x t); one transpose per PSUM bank.
                # hi parts per head; lo parts packed 4-heads-per-transpose.
                athi = wpool.tile([128, H, 128], F16, tag="athi",
                                  name=f"athi_{i}")
                atlo4 = wpool.tile([128, 2, 128], F16, tag="atlo4",
                                   name=f"atlo4_{i}")
                for r in range(4):          # head pairs: transposes
                    h0, h1 = 2 * r, 2 * r + 1
                    trhi = papool.tile([128, 2, 1024], F16, tag="pa",
                                       name=f"trhi_{i}_{r}")
                    for hh, h in ((0, h0), (1, h1)):
                        nc.tensor.transpose(
                            trhi[:, hh, 0:128],
                            aband[:, h * 128:h * 128 + 128], idf_s[:])
                    nc.vector.tensor_copy(athi[:, h0:h0 + 2, :],
                                          trhi[:, :, 0:128])
                trlo = papool.tile([128, 2, 1024], F16, tag="pa",
                                   name=f"trlo_{i}")
                for g_ in range(2):
                    nc.tensor.transpose(
                        trlo[:, g_, 0:128],
                        aband[:, 1024 + g_ * 128:1024 + g_ * 128 + 128],
                        idf_s[:])
                nc.scalar.copy(atlo4[:], trlo[:, :, 0:128])
                # replicate V lo rows to all four 32-partition groups
                pvr = pbpool.tile([128, 2, D], F32, tag="pb",
                                  name=f"pvr_{i}")
                nc.tensor.matmul(pvr[:, 0, :], rep_s[:],
                                 v_s[0:32, i + 1, :], start=True, stop=True)
                vrep = wpool.tile([128, D], F16, tag="vrep", name=f"vrep_{i}")
                nc.vector.tensor_copy(vrep[:], pvr[:, 0, :])

                osb = wpool.tile([128, D], F32, tag="osb", name=f"osb_{i}")
                for r in range(4):          # head pairs: out matmuls
                    h0, h1 = 2 * r, 2 * r + 1
                    po = pbpool.tile([128, 2, D], F32, tag="pb",
                                     name=f"po_{i}_{r}")
                    for hh, h in ((0, h0), (1, h1)):
                        g_, sl = h // 4, h % 4
                        nc.tensor.matmul(
                            po[:, hh, 0:HD], athi[:, h, :],
                            v_s[:, i, h * HD:(h + 1) * HD],
                            start=True, stop=False)
                        nc.tensor.matmul(
                            po[:, hh, 0:HD],
                            atlo4[32 * sl:32 * sl + 32, g_, :],
                            vrep[32 * sl:32 * sl + 32, h * HD:(h + 1) * HD],
                            start=False, stop=True,
                            tile_position=(32 * sl, 0))
                    if r % 2 == 0:
                        nc.vector.tensor_copy(
                            osb[:, h0 * HD:(h0 + 2) * HD].rearrange(
                                "p (b c) -> p b c", b=2),
                            po[:, :, 0:HD])
                    else:
                        nc.scalar.copy(
                            osb[:, h0 * HD:(h0 + 2) * HD].rearrange(
                                "p (b c) -> p b c", b=2),
                            po[:, :, 0:HD])
                nc.sync.dma_start(out_d.ap()[t0:t0 + 128, :], osb[:])

            # ---- interleaved emission: projections lead attention by 1 group
            for g in range(NSC):
                if g == 0:
                    emit_qk_proj(1, 0)      # k first (attention needs halo)
                    emit_qk_proj(1, 1)
                    emit_qk_proj(0, 0)
                    emit_late_consts()
                elif g + 1 < NSC:
                    emit_qk_proj(1, g + 1)
                if g + 1 < NSC:
                    emit_qk_proj(0, g + 1)
                while v_emitted <= min(4 * g + 4, NVC - 1):
                    emit_v_proj(v_emitted)
                    v_emitted += 1
                for i in range(4 * g, 4 * g + 4):
                    emit_attn_front(i)
                    if i >= LAG:
                        emit_attn_back(i - LAG)
            for i in range(NT - LAG, NT):
                emit_attn_back(i)

    nc.compile()
    return nc


def _host_prepare(inputs, layer):
    d, r0, cols, span, SW, PADLO = _plan(layer)
    q = np.ascontiguousarray(np.asarray(inputs["query"], np.float32))
    k = np.ascontiguousarray(np.asarray(inputs["key"], np.float32))
    v = np.ascontiguousarray(np.asarray(inputs["value"], np.float32))
    Wq = np.asarray(inputs["Wq"], np.float32)
    Wk = np.asarray(inputs["Wk"], np.float32)
    Wv = np.asarray(inputs["Wv"], np.float32)
    Er = np.asarray(inputs["Er"], np.float32)

    def wstack(Wm):
        return np.ascontiguousarray(
            Wm.T.reshape(4, 128, D).transpose(1, 0, 2).astype(np.float16))

    consts = {
        "WqTs": wstack(Wq), "WkTs": wstack(Wk), "WvTs": wstack(Wv),
        "bqc": np.ascontiguousarray(
            np.asarray(inputs["bq"], np.float32).reshape(4, 128).T),
        "bkc": np.ascontiguousarray(
            np.asarray(inputs["bk"], np.float32).reshape(4, 128).T),
        "bv1": np.asarray(inputs["bv"], np.float16).reshape(1, D),
    }
    eones = np.ones((1, 3 * 128), np.float16)
    eones[0, 0:PADLO] = 0.0
    NVC = NT + 1
    tlo_last = (NVC - 1) * 128 - PADLO
    nvalid = max(0, T - tlo_last)
    eones[0, 2 * 128:] = 0.0
    eones[0, 2 * 128:2 * 128 + nvalid] = 1.0
    consts["eones"] = eones

    erk = np.zeros((128, 4, NT, 6), np.float16)
    for h in range(H):
        dp, half = h // 2, h % 2
        blk = Er[h, :, ::-1].astype(np.float16)        # (64, 6) device order
        erk[half * 64:(half + 1) * 64, dp, :, :] = blk[:, None, :]
    consts["ErK"] = erk

    idx = np.zeros((128, 48), np.int16)
    for p in range(128):
        for h in range(H):
            for j in range(6):
                c = p + cols[j]
                if c < 128:
                    dst = h * 128 + c
                else:
                    dst = 1024 + (h // 4) * 128 + (h % 4) * 32 + (c - 128)
                idx[p, h * 6 + j] = dst
    consts["idx16"] = idx
    rep = np.zeros((32, 128), np.float16)
    for p in range(32):
        rep[p, p::32] = 1.0
    consts["rep16"] = rep
    consts["idf16"] = np.eye(128, dtype=np.float16)

    in_maps = []
    for c in range(B):
        m = dict(consts)
        m["qT"] = np.ascontiguousarray(q[c].T.astype(np.float16))
        m["kTi"] = np.ascontiguousarray(k[c].T.astype(np.float16))
        m["vT"] = np.ascontiguousarray(v[c].T.astype(np.float16))
        in_maps.append(m)
    return in_maps


def kernel(**inputs):
    layer = int(np.asarray(inputs["layer"]))
    if layer not in _CACHE:
        _CACHE[layer] = _build(layer)
    nc = _CACHE[layer]
    in_maps = _host_prepare(inputs, layer)
    res = bass_utils.run_bass_kernel_spmd(nc, in_maps, core_ids=list(range(B)))
    outs, attns = [], []
    for c in range(B):
        r = res.results[c]
        outs.append(np.asarray(r["out"], np.float32))
        ac = np.asarray(r["attnc"], np.float32).reshape(T, H, 6)
        attns.append(ac.transpose(1, 0, 2)[:, :, ::-1][:, :, None, :])
    return np.stack(outs), np.stack(attns)


# revision 28
# speedup vs baseline: 1.0752x; 1.0125x over previous
"""Trainium2 Bass kernel for dilated multi-head self-attention with relative
positional embedding (B=8 data-parallel over 8 NeuronCores, 1 sample/core).

Pipeline per core:
  - q/k projections on PE (W-stationary). k^T is stored in a per-query-tile
    block layout kT2[(d), tile, 160]: cols 0:147 = the sliding key window for
    that tile (halos duplicated), cols 147:153 = Er columns for the head that
    owns the partition range. One fp16 matmul per (tile, head) then yields the
    full banded scores + q@Er in a single PSUM bank region.
  - v projection (x^T-stationary) producing V in (t x d) layout shifted by the
    window lower bound so out-matmul key segments are partition-aligned.
  - banded scores written fp16 to DRAM scratch; strided DMA readback of the
    per-row rolled window (row stride = pitch+1 turns the band diagonal into
    a column) -> compact (t x 6) taps + qEr.
  - compact softmax; normalized attention in fp32 (graded output) and fp16.
  - GPSIMD local_scatter rebuilds the banded attention matrix (t x k, fp16,
    zeros off-band), PE transposes -> (k x t), PE matmul against V segments.

Hardware constraint honored throughout: at most ONE matmul output region per
PSUM bank (multiple different-region matmuls into one bank crash the device);
same-region accumulation groups are fine.
"""

import math
import sys

sys.path.insert(0, "/opt/trn_rl_repo")

import numpy as np

from concourse import bass, bacc, mybir, tile
from concourse import bass_utils

DT = mybir.dt
F32 = DT.float32
F16 = DT.float16
I16 = DT.int16

B, T, D = 8, 4096, 512
H, HD = 8, 64
NT = T // 128              # 32 query tiles
NSC = 8                    # projection super-chunks
SCW = T // NSC             # 512
KBW = 160                  # kT2 block width: 147 band + 6 Er + pad
PITCH = 160                # DRAM band row pitch (elems)
ABW = 160                  # A-band width (128 + 32)
LAG = 6                    # tiles of PE-stream lag hiding the DMA roundtrip

_CACHE = {}


def _plan(layer):
    d = 2 ** layer
    r0 = int(round((1 + layer) / 11 * 10))
    cols = [j * d for j in range(5)] + [4 * d + r0]   # device tap cols (ascending)
    span = 4 * d + r0                                  # 19 @ layer=2
    sw = 128 + span                                    # 147 band columns
    padlo = 4 * d                                      # 16
    assert sw + 6 <= min(PITCH, KBW) and span < 32 and padlo < 128
    return d, r0, cols, span, sw, padlo


def _build(layer):
    d, r0, cols, span, SW, PADLO = _plan(layer)
    NVC = NT + 1                          # V chunks (33): rows r = t + PADLO
    ROLLW = span + 1                      # 20

    nc = bacc.Bacc("TRN2", target_bir_lowering=False, debug=False)

    # ---- DRAM I/O ----
    qT_d = nc.dram_tensor("qT", (D, T), F16, kind="ExternalInput")
    kTi_d = nc.dram_tensor("kTi", (D, T), F16, kind="ExternalInput")
    vT_d = nc.dram_tensor("vT", (D, T), F16, kind="ExternalInput")
    wq_d = nc.dram_tensor("WqTs", (128, 4, D), F16, kind="ExternalInput")
    wk_d = nc.dram_tensor("WkTs", (128, 4, D), F16, kind="ExternalInput")
    wv_d = nc.dram_tensor("WvTs", (128, 4, D), F16, kind="ExternalInput")
    bqc_d = nc.dram_tensor("bqc", (128, 4), F32, kind="ExternalInput")
    bkc_d = nc.dram_tensor("bkc", (128, 4), F32, kind="ExternalInput")
    bv1_d = nc.dram_tensor("bv1", (1, D), F16, kind="ExternalInput")
    eones_d = nc.dram_tensor("eones", (1, 3 * 128), F16, kind="ExternalInput")
    erk_d = nc.dram_tensor("ErK", (128, 4, NT, 6), F16, kind="ExternalInput")
    idx_d = nc.dram_tensor("idx16", (128, 48), I16, kind="ExternalInput")
    idf_d = nc.dram_tensor("idf16", (128, 128), F16, kind="ExternalInput")
    rep_d = nc.dram_tensor("rep16", (32, 128), F16, kind="ExternalInput")

    out_d = nc.dram_tensor("out", (T, D), F32, kind="ExternalOutput")
    attnc_d = nc.dram_tensor("attnc", (NT, 128, 48), F32, kind="ExternalOutput")

    band_d = nc.dram_tensor("bandscr", (NT, H, 128, PITCH), F16, kind="Internal")
    warm_d = nc.dram_tensor("warmscr", (128, 64), F16, kind="Internal")

    scale = 1.0 / math.sqrt(HD)

    with tile.TileContext(nc) as tc:
        with (
            tc.tile_pool(name="const", bufs=1) as cpool,
            tc.tile_pool(name="persist", bufs=1) as ppool,
            tc.tile_pool(name="qring", bufs=2) as qpool,
            tc.tile_pool(name="xin", bufs=3) as xpool,
            tc.tile_pool(name="vin", bufs=3) as vpool,
            tc.tile_pool(name="pA", bufs=2, space="PSUM") as papool,
            tc.tile_pool(name="pB", bufs=2, space="PSUM") as pbpool,
            tc.tile_pool(name="work", bufs=3) as wpool,
        ):
            # ---- HAM warmup: ~9us of dense dummy matmuls, no DMA deps ----
            wrm = cpool.tile([128, 512], F16, tag="wrm")
            nc.vector.memset(wrm[:], 0.0)
            pwarm = papool.tile([128, 2, 512], F32, tag="pa", name="pwarm")
            for w_ in range(64):
                nc.tensor.matmul(pwarm[:, 0, :], wrm[:, 0:128], wrm[:],
                                 start=(w_ == 0), stop=(w_ == 63))
            wrm2 = cpool.tile([128, 64], F16, tag="wrm2")
            nc.scalar.copy(wrm2[:], pwarm[:, 0, 0:64])
            nc.sync.dma_start(warm_d.ap(), wrm2[:])

            # ---- constants ----
            wq_s = cpool.tile([128, 4, D], F16, tag="wq")
            wk_s = cpool.tile([128, 4, D], F16, tag="wk")
            wv_s = cpool.tile([128, 4, D], F16, tag="wv")
            bqc_s = cpool.tile([128, 4], F32, tag="bqc")
            bkc_s = cpool.tile([128, 4], F32, tag="bkc")
            bv1_s = cpool.tile([1, D], F16, tag="bv1")
            eones_s = cpool.tile([1, 3 * 128], F16, tag="eones")
            idx_s = cpool.tile([128, 48], I16, tag="idx")
            idf_s = cpool.tile([128, 128], F16, tag="idf")
            rep_s = cpool.tile([32, 128], F16, tag="rep")
            for t_, d_ in ((wk_s, wk_d), (bkc_s, bkc_d), (wq_s, wq_d),
                           (bqc_s, bqc_d)):
                nc.sync.dma_start(t_[:], d_.ap())

            # ---- persistent activations ----
            kT2 = ppool.tile([128, 4, NT, KBW], F16, tag="kT2")
            v_s = ppool.tile([128, NVC, D], F16, tag="V")
            # zero edges: block 0 left halo, block NT-1 right halo, pad col
            nc.vector.memset(kT2[:, :, 0, 0:PADLO], 0.0)
            nc.vector.memset(kT2[:, :, NT - 1, PADLO + 128:SW], 0.0)

            # Er columns (constant per (partition-range, head))
            for dp in range(4):
                nc.sync.dma_start(kT2[:, dp, :, SW:SW + 6],
                                  erk_d.ap()[:, dp])

            def emit_late_consts():
                for t_, d_ in ((wv_s, wv_d), (bv1_s, bv1_d),
                               (eones_s, eones_d), (idx_s, idx_d),
                               (idf_s, idf_d), (rep_s, rep_d)):
                    nc.sync.dma_start(t_[:], d_.ap())

            qtiles = [None] * NSC
            v_emitted = 0

            def emit_qk_proj(proj, sc):
                xd = qT_d if proj == 0 else kTi_d
                w_s = wq_s if proj == 0 else wk_s
                bc_s = bqc_s if proj == 0 else bkc_s
                xin = xpool.tile([128, 4, SCW], F16, tag="xin",
                                 name=f"xin_{proj}_{sc}")
                nc.sync.dma_start(xin[:], bass.AP(
                    tensor=xd, offset=sc * SCW,
                    ap=[[T, 128], [128 * T, 4], [1, SCW]]))
                if proj == 0:
                    qt = qpool.tile([128, 4, SCW], F16, tag="q", name=f"q_{sc}")
                    qtiles[sc] = qt
                for oc in range(4):
                    ps = pbpool.tile([128, 2, SCW], F32, tag="pb",
                                     name=f"pspj_{proj}_{sc}_{oc}")
                    for kc in range(4):
                        nc.tensor.matmul(
                            ps[:, 0, :], w_s[:, kc, oc * 128:(oc + 1) * 128],
                            xin[:, kc, :], start=(kc == 0), stop=(kc == 3))
                    if proj == 0:
                        nc.vector.tensor_scalar_add(
                            qtiles[sc][:, oc, :], ps[:, 0, :], bc_s[:, oc:oc + 1])
                    else:
                        # scatter into kT2 block layout (f16, bias fused)
                        j0 = 4 * sc
                        psb = ps[:, 0, :].rearrange("p (b c) -> p b c", b=4)
                        nc.vector.tensor_scalar_add(
                            kT2[:, oc, j0:j0 + 4, PADLO:PADLO + 128],
                            psb, bc_s[:, oc:oc + 1])
                        # left halos -> blocks j0+1 .. j0+4
                        nb = 3 if sc == NSC - 1 else 4
                        nc.vector.tensor_scalar_add(
                            kT2[:, oc, j0 + 1:j0 + 1 + nb, 0:PADLO],
                            psb[:, 0:nb, 128 - PADLO:128],
                            bc_s[:, oc:oc + 1])
                        # right halos -> blocks j0-1 .. j0+2
                        hb = span - PADLO      # 3 cols
                        if sc == 0:
                            nc.vector.tensor_scalar_add(
                                kT2[:, oc, 0:3, PADLO + 128:SW],
                                psb[:, 1:4, 0:hb], bc_s[:, oc:oc + 1])
                        else:
                            nc.vector.tensor_scalar_add(
                                kT2[:, oc, j0 - 1:j0 + 3, PADLO + 128:SW],
                                psb[:, 0:4, 0:hb], bc_s[:, oc:oc + 1])

            def emit_v_proj(j):
                tlo = j * 128 - PADLO
                vin = vpool.tile([128, 4, 128], F16, tag="vin", name=f"vin_{j}")
                lo_clip = max(0, -tlo)
                hi_clip = max(0, tlo + 128 - T)
                if lo_clip or hi_clip:
                    nc.vector.memset(vin[:], 0.0)
                nrow = 128 - lo_clip - hi_clip
                nc.sync.dma_start(
                    vin[:, :, lo_clip:lo_clip + nrow],
                    bass.AP(tensor=vT_d, offset=tlo + lo_clip,
                            ap=[[T, 128], [128 * T, 4], [1, nrow]]))
                pv = pbpool.tile([128, 2, D], F32, tag="pb", name=f"pspv_{j}")
                for kc in range(4):
                    nc.tensor.matmul(
                        pv[:, 0, :], vin[:, kc, :], wv_s[:, kc, :],
                        start=(kc == 0), stop=False)
                ev = 0 if j == 0 else (2 if j == NVC - 1 else 1)
                nc.tensor.matmul(
                    pv[:, 0, :], eones_s[0:1, ev * 128:(ev + 1) * 128], bv1_s[:],
                    start=False, stop=True)
                nc.scalar.copy(v_s[:, j, :], pv[:, 0, :])

            abands = [None] * NT

            def emit_attn_front(i):
                t0 = 128 * i
                sc, scoff = i // 4, (i % 4) * 128
                qt = qtiles[sc]
                band = wpool.tile([128, H, PITCH], F16, tag="band",
                                  name=f"band_{i}")
                for g in range(4):          # head pairs; 1 matmul per bank
                    ps = papool.tile([128, 2, 512], F32, tag="pa",
                                     name=f"psbd_{i}_{g}")
                    for hh in range(2):
                        h = 2 * g + hh
                        dp, bp = h // 2, (h % 2) * 64
                        nc.tensor.matmul(
                            ps[:, hh, 0:SW + 6],
                            qt[bp:bp + 64, dp, scoff:scoff + 128],
                            kT2[bp:bp + 64, dp, i, 0:SW + 6],
                            start=True, stop=True)
                    if g % 2 == 0:
                        nc.scalar.copy(band[:, 2 * g:2 * g + 2, 0:SW + 6],
                                       ps[:, :, 0:SW + 6])
                    else:
                        nc.vector.tensor_copy(band[:, 2 * g:2 * g + 2, 0:SW + 6],
                                              ps[:, :, 0:SW + 6])
                nc.sync.dma_start(
                    bass.AP(tensor=band_d, offset=i * (H * 128 * PITCH),
                            ap=[[PITCH, 128], [128 * PITCH, H], [1, SW + 6]]),
                    band[:, :, 0:SW + 6])

                # rolled readback: roll[p,h,c] = band[i,h,p,p+c]
                roll = wpool.tile([128, H, ROLLW], F16, tag="roll",
                                  name=f"roll_{i}")
                base = i * (H * 128 * PITCH)
                nc.sync.dma_start(roll[:], bass.AP(
                    tensor=band_d, offset=base,
                    ap=[[PITCH + 1, 128], [128 * PITCH, H], [1, ROLLW]]))
                er2 = wpool.tile([128, H, 6], F16, tag="er2", name=f"er2_{i}")
                nc.sync.dma_start(er2[:], bass.AP(
                    tensor=band_d, offset=base + SW,
                    ap=[[PITCH, 128], [128 * PITCH, H], [1, 6]]))

                # compact softmax
                sc48 = wpool.tile([128, H, 6], F32, tag="sc48", name=f"sc48_{i}")
                nc.vector.tensor_add(
                    sc48[:, :, 0:5], roll[:, :, 0:4 * d + 1:d], er2[:, :, 0:5])
                nc.vector.tensor_add(
                    sc48[:, :, 5:6], roll[:, :, 4 * d + r0:4 * d + r0 + 1],
                    er2[:, :, 5:6])
                e48 = wpool.tile([128, H, 6], F32, tag="e48", name=f"e48_{i}")
                nc.scalar.activation(
                    e48[:], sc48[:], mybir.ActivationFunctionType.Exp,
                    scale=scale)
                sums = wpool.tile([128, H, 1], F32, tag="sums", name=f"sums_{i}")
                nc.vector.reduce_sum(sums[:, :, 0], e48[:],
                                     axis=mybir.AxisListType.X)
                n8 = wpool.tile([128, H, 1], F32, tag="n8", name=f"n8_{i}")
                nc.vector.reciprocal(n8[:], sums[:])
                a48 = wpool.tile([128, H, 6], F32, tag="a48", name=f"a48_{i}")
                nc.vector.tensor_mul(a48[:], e48[:],
                                     n8[:].broadcast_to([128, H, 6]))
                nc.sync.dma_start(attnc_d.ap()[i],
                                  a48[:].rearrange("p h c -> p (h c)"))
                a48h = wpool.tile([128, 48], F16, tag="a48h", name=f"a48h_{i}")
                nc.vector.tensor_copy(a48h[:],
                                      a48[:].rearrange("p h c -> p (h c)"))

                # scatter into banded A (t x k), fp16
                aband = wpool.tile([128, H * ABW], F16, tag="aband",
                                   name=f"aband_{i}", bufs=LAG + 2)
                abands[i] = aband
                nc.gpsimd.local_scatter(
                    aband[:], a48h[:], idx_s[:],
                    channels=128, num_elems=H * ABW, num_idxs=48)

            def emit_attn_back(i):
                t0 = 128 * i
                aband = abands[i]
                # transposes -> (k x t); one transpose per PSUM bank.
                # hi parts per head; lo parts packed 4-heads-per-transpose.
                athi = wpool.tile([128, H, 128], F16, tag="athi",
                                  name=f"athi_{i}")
                atlo4 = wpool.tile([128, 2, 128], F16, tag="atlo4",
                                   name=f"atlo4_{i}")
                for r in range(4):          # head pairs: transposes
                    h0, h1 = 2 * r, 2 * r + 1
                    trhi = papool.tile([128, 2, 1024], F16, tag="pa",
                                       name=f"trhi_{i}_{r}")
                    for hh, h in ((0, h0), (1, h1)):
                        nc.tensor.transpose(
                            trhi[:, hh, 0:128],
                            aband[:, h * 128:h * 128 + 128], idf_s[:])
                    nc.vector.tensor_copy(athi[:, h0:h0 + 2, :],
                                          trhi[:, :, 0:128])
                trlo = papool.tile([128, 2, 1024], F16, tag="pa",
                                   name=f"trlo_{i}")
                for g_ in range(2):
                    nc.tensor.transpose(
                        trlo[:, g_, 0:128],
                        aband[:, 1024 + g_ * 128:1024 + g_ * 128 + 128],
                        idf_s[:])
                nc.scalar.copy(atlo4[:], trlo[:, :, 0:128])
                # replicate V lo rows to all four 32-partition groups
                pvr = pbpool.tile([128, 2, D], F32, tag="pb",
                                  name=f"pvr_{i}")
                nc.tensor.matmul(pvr[:, 0, :], rep_s[:],
                                 v_s[0:32, i + 1, :], start=True, stop=True)
                vrep = wpool.tile([128, D], F16, tag="vrep", name=f"vrep_{i}")
                nc.vector.tensor_copy(vrep[:], pvr[:, 0, :])

                osb = wpool.tile([128, D], F32, tag="osb", name=f"osb_{i}")
                for r in range(4):          # head pairs: out matmuls
                    h0, h1 = 2 * r, 2 * r + 1
                    po = pbpool.tile([128, 2, D], F32, tag="pb",
                                     name=f"po_{i}_{r}")
                    for hh, h in ((0, h0), (1, h1)):
                        g_, sl = h // 4, h % 4
                        nc.tensor.matmul(
                            po[:, hh, 0:HD], athi[:, h, :],
                            v_s[:, i, h * HD:(h + 1) * HD],
                            start=True, stop=False)
                        nc.tensor.matmul(
                            po[:, hh, 0:HD],
                            atlo4[32 * sl:32 * sl + 32, g_, :],
                            vrep[32 * sl:32 * sl + 32, h * HD:(h + 1) * HD],
                            start=False, stop=True,
                            tile_position=(32 * sl, 0))
                    if r % 2 == 0:
                        nc.vector.tensor_copy(
                            osb[:, h0 * HD:(h0 + 2) * HD].rearrange(
                                "p (b c) -> p b c", b=2),
                            po[:, :, 0:HD])
                    else:
                        nc.scalar.copy(
                            osb[:, h0 * HD:(h0 + 2) * HD].rearrange(
                                "p (b c) -> p b c", b=2),
                            po[:, :, 0:HD])
                nc.sync.dma_start(out_d.ap()[t0:t0 + 128, :], osb[:])

            # ---- interleaved emission: projections lead attention by 1 group
            for g in range(NSC):
                if g == 0:
                    emit_qk_proj(1, 0)      # k first (attention needs halo)
                    emit_qk_proj(1, 1)
                    emit_qk_proj(0, 0)
                    emit_late_consts()
                elif g + 1 < NSC:
                    emit_qk_proj(1, g + 1)
                if g + 1 < NSC:
                    emit_qk_proj(0, g + 1)
                while v_emitted <= min(4 * g + 4, NVC - 1):
                    emit_v_proj(v_emitted)
                    v_emitted += 1
                for i in range(4 * g, 4 * g + 4):
                    emit_attn_front(i)
                    if i >= LAG:
                        emit_attn_back(i - LAG)
            for i in range(NT - LAG, NT):
                emit_attn_back(i)

    nc.compile()
    return nc


def _host_prepare(inputs, layer):
    d, r0, cols, span, SW, PADLO = _plan(layer)
    q = np.ascontiguousarray(np.asarray(inputs["query"], np.float32))
    k = np.ascontiguousarray(np.asarray(inputs["key"], np.float32))
    v = np.ascontiguousarray(np.asarray(inputs["value"], np.float32))
    Wq = np.asarray(inputs["Wq"], np.float32)
    Wk = np.asarray(inputs["Wk"], np.float32)
    Wv = np.asarray(inputs["Wv"], np.float32)
    Er = np.asarray(inputs["Er"], np.float32)

    def wstack(Wm):
        return np.ascontiguousarray(
            Wm.T.reshape(4, 128, D).transpose(1, 0, 2).astype(np.float16))

    consts = {
        "WqTs": wstack(Wq), "WkTs": wstack(Wk), "WvTs": wstack(Wv),
        "bqc": np.ascontiguousarray(
            np.asarray(inputs["bq"], np.float32).reshape(4, 128).T),
        "bkc": np.ascontiguousarray(
            np.asarray(inputs["bk"], np.float32).reshape(4, 128).T),
        "bv1": np.asarray(inputs["bv"], np.float16).reshape(1, D),
    }
    eones = np.ones((1, 3 * 128), np.float16)
    eones[0, 0:PADLO] = 0.0
    NVC = NT + 1
    tlo_last = (NVC - 1) * 128 - PADLO
    nvalid = max(0, T - tlo_last)
    eones[0, 2 * 128:] = 0.0
    eones[0, 2 * 128:2 * 128 + nvalid] = 1.0
    consts["eones"] = eones

    erk = np.zeros((128, 4, NT, 6), np.float16)
    for h in range(H):
        dp, half = h // 2, h % 2
        blk = Er[h, :, ::-1].astype(np.float16)        # (64, 6) device order
        erk[half * 64:(half + 1) * 64, dp, :, :] = blk[:, None, :]
    consts["ErK"] = erk

    idx = np.zeros((128, 48), np.int16)
    for p in range(128):
        for h in range(H):
            for j in range(6):
                c = p + cols[j]
                if c < 128:
                    dst = h * 128 + c
                else:
                    dst = 1024 + (h // 4) * 128 + (h % 4) * 32 + (c - 128)
                idx[p, h * 6 + j] = dst
    consts["idx16"] = idx
    rep = np.zeros((32, 128), np.float16)
    for p in range(32):
        rep[p, p::32] = 1.0
    consts["rep16"] = rep
    consts["idf16"] = np.eye(128, dtype=np.float16)

    in_maps = []
    for c in range(B):
        m = dict(consts)
        m["qT"] = np.ascontiguousarray(q[c].T.astype(np.float16))
        m["kTi"] = np.ascontiguousarray(k[c].T.astype(np.float16))
        m["vT"] = np.ascontiguousarray(v[c].T.astype(np.float16))
        in_maps.append(m)
    return in_maps


def kernel(**inputs):
    layer = int(np.asarray(inputs["layer"]))
    if layer not in _CACHE:
        _CACHE[layer] = _build(layer)
    nc = _CACHE[layer]
    in_maps = _host_prepare(inputs, layer)
    res = bass_utils.run_bass_kernel_spmd(nc, in_maps, core_ids=list(range(B)))
    outs, attns = [], []
    for c in range(B):
        r = res.results[c]
        outs.append(np.asarray(r["out"], np.float32))
        ac = np.asarray(r["attnc"], np.float32).reshape(T, H, 6)
        attns.append(ac.transpose(1, 0, 2)[:, :, ::-1][:, :, None, :])
    return np.stack(outs), np.stack(attns)
